# revision 13
# baseline (speedup 1.0000x reference)
"""BiLSTM + CRF (Viterbi) Trainium2 kernel.

Strategy:
  - Data-parallel over batch: 8 NeuronCores x 8 sequences each.
  - Each core runs the full 2-layer bidirectional LSTM for its 8 sequences:
    4 sub-phases (L0 fwd, L0 bwd, L1 fwd, L1 bwd), each a For_i loop over 64
    blocks of 8 timesteps.
  - Input projections (x @ W_ih.T) are hoisted per block as big matmuls with
    the (transposed) activations as the stationary operand; the recurrent
    h @ W_hh.T runs per step with h.T stationary and W_hh.T streaming.
  - All matmuls in fp32 (PE 4 cyc/row) so features match the fp32 reference
    closely enough that every Viterbi argmax decision is preserved.
  - The backward direction consumes host-reversed token indices; layer 1
    reads the opposite direction's h-history with a reversed access pattern.
  - Device emits per-(t,b) tag features (split by direction); the Viterbi DP
    runs on the host in fp32 numpy, which is bit-exact vs the jax reference
    (the DP uses only 2-operand adds and max reductions).
"""

import os
import sys

for _p in ("/opt/trn_rl_repo", "/root/.axon_site/_ro/trn_rl_repo"):
    if os.path.isdir(_p) and _p not in sys.path:
        sys.path.insert(0, _p)

# The device path needs the axon/neuron jax backend; a JAX_PLATFORMS=cpu pin
# (commonly used for references) would hide the NeuronCores.
if os.environ.get("JAX_PLATFORMS") == "cpu" and "jax" not in sys.modules:
    os.environ["JAX_PLATFORMS"] = ""

import numpy as np

C = 16
START = 14
STOP = 15
NEG = -10000.0
B, T, V, E, H = 64, 512, 50000, 512, 1024
HD = H // 2
NCORES = 8
BL = B // NCORES          # sequences per core
SPB = 16                  # timesteps per block
NBLK = T // SPB           # 32 blocks of 16 steps x 8 seqs = 128 tokens

_BUILD_CACHE: dict = {}


def _build_program(T=T, V=V, NCORES=NCORES, phases=((0,'f'),(0,'b'),(1,'f'),(1,'b')), dbg=()):
    """Build the single-core SPMD Bass program (same NEFF on all cores)."""
    NBLK = T // SPB
    import concourse.bass as bass
    import concourse.mybir as mybir
    import concourse.tile as tile
    from concourse import bacc
    from concourse.bass import ds, ts
    from concourse.masks import make_identity

    fp32 = mybir.dt.float32
    i32 = mybir.dt.int32
    AF = mybir.ActivationFunctionType
    OP = mybir.AluOpType

    nc = bacc.Bacc("TRN2", target_bir_lowering=False, debug=False,
                   num_devices=NCORES)

    # ---- external inputs -------------------------------------------------
    emb_d = nc.dram_tensor("emb", [V, E], fp32, kind="ExternalInput")
    tok_d = {}
    for dr in ("f", "b"):
        tok_d[dr] = nc.dram_tensor(f"tok{dr}", [T * BL, 1], i32,
                                   kind="ExternalInput")
    wih_d, whh_d, bias_d = {}, {}, {}
    for lay in (0, 1):
        kin = E if lay == 0 else H
        for dr in ("f", "b"):
            wih_d[lay, dr] = nc.dram_tensor(f"wih{lay}{dr}", [kin, 4 * HD],
                                            fp32, kind="ExternalInput")
            whh_d[lay, dr] = nc.dram_tensor(f"whh{lay}{dr}", [HD, 4 * HD],
                                            fp32, kind="ExternalInput")
            bias_d[lay, dr] = nc.dram_tensor(f"bias{lay}{dr}", [128, 4 * HD],
                                             fp32, kind="ExternalInput")
    h0_d = nc.dram_tensor("h0T", [4, HD, BL], fp32, kind="ExternalInput")
    c0_d = nc.dram_tensor("c0v", [4, BL, HD], fp32, kind="ExternalInput")
    linw_d = nc.dram_tensor("linwT", [H, C], fp32, kind="ExternalInput")

    # ---- external outputs ------------------------------------------------
    fo_d = {dr: nc.dram_tensor(f"fo{dr}", [NBLK, C, SPB * BL], fp32,
                               kind="ExternalOutput") for dr in ("f", "b")}

    # ---- internal DRAM h.T histories ------------------------------------
    # layout: [blk][kchunk(4)][hd 128][t(8) * b(8)]
    hist_d = {dr: nc.dram_tensor(f"hist{dr}", [NBLK, 4, 128, SPB * BL], fp32,
                                 kind="Internal") for dr in ("f", "b")}
    # same content with the within-block time order reversed, so the peer
    # direction can read it with positive strides only
    histr_d = {dr: nc.dram_tensor(f"histr{dr}", [NBLK, 4, 128, SPB * BL], fp32,
                                  kind="Internal") for dr in ("f", "b")}

    emb_ap = emb_d.ap()

    with tile.TileContext(nc) as tc:
        with tc.tile_pool(name="persist", bufs=1) as pers:
            ident = pers.tile([128, 128], fp32)
            make_identity(nc, ident[:])
            linw_sb = pers.tile([128, 8, C], fp32)
            nc.sync.dma_start(linw_sb[:], linw_d.ap().rearrange(
                "(c p) n -> p c n", p=128))

            def lstm_phase(lay, dr):
                """Emit one LSTM direction pass (64-block For_i loop)."""
                kin = E if lay == 0 else H
                kc = kin // 128           # stationary K chunks (4 or 8)
                run = {(0, "f"): 0, (0, "b"): 1, (1, "f"): 2, (1, "b"): 3}[lay, dr]
                with tc.tile_pool(name=f"sb{lay}{dr}", bufs=1) as sb, \
                     tc.tile_pool(name=f"io{lay}{dr}", bufs=2) as io, \
                     tc.tile_pool(name=f"ps{lay}{dr}", bufs=1, space="PSUM") as ps, \
                     tc.tile_pool(name=f"ps2{lay}{dr}", bufs=2, space="PSUM") as ps2:
                    # weights / bias resident
                    wih_t = sb.tile([128, kc, 4 * HD], fp32)
                    nc.sync.dma_start(wih_t[:], wih_d[lay, dr].ap().rearrange(
                        "(c p) n -> p c n", p=128))
                    whh_t = sb.tile([128, 4, 4 * HD], fp32)
                    nc.sync.dma_start(whh_t[:], whh_d[lay, dr].ap().rearrange(
                        "(c p) n -> p c n", p=128))
                    bias_t = sb.tile([128, 4 * HD], fp32)
                    nc.sync.dma_start(bias_t[:], bias_d[lay, dr].ap())
                    # state
                    hA = sb.tile([128, 4, BL], fp32)
                    hB = sb.tile([128, 4, BL], fp32)
                    nc.sync.dma_start(hA[:], h0_d.ap()[run].rearrange(
                        "(c p) m -> p c m", p=128))
                    c_t = sb.tile([BL, 4, 128], fp32)
                    nc.sync.dma_start(c_t[:], c0_d.ap()[run].rearrange(
                        "m (c p) -> m c p", p=128))

                    with tc.For_i(0, NBLK) as blk:
                        # ---- stationary activations for this block ------
                        if lay == 0:
                            idx = io.tile([128, 1], i32, tag="idx")
                            nc.sync.dma_start(
                                idx[:], tok_d[dr].ap()[ds(blk * 128, 128)])
                            xg = io.tile([128, E], fp32, tag="xg")
                            nc.gpsimd.indirect_dma_start(
                                out=xg[:], out_offset=None, in_=emb_ap,
                                in_offset=bass.IndirectOffsetOnAxis(
                                    ap=idx[:, :1], axis=0))
                            xT = io.tile([128, kc, 128], fp32, tag="xT")
                            for k in range(kc):
                                tp = ps.tile([128, 128], fp32, tag="xtp")
                                nc.tensor.transpose(
                                    tp[:], xg[:, ts(k, 128)], ident[:])
                                nc.vector.tensor_copy(xT[:, k], tp[:])
                        else:
                            # load h histories (own straight, peer reversed)
                            # layer-1 input is [hf | hb]; the own-direction
                            # history reads straight, the peer's reads via the
                            # pre-reversed copy with a flipped block index.
                            own = hist_d[dr]
                            peer = histr_d["b" if dr == "f" else "f"]
                            own_ap = own.ap()[ds(blk, 1)].rearrange(
                                "x k p f -> p (x k) f")
                            peer_ap = peer.ap()[ds(NBLK - 1 - blk, 1)].rearrange(
                                "x k p f -> p (x k) f")
                            xT = io.tile([128, kc, 128], fp32, tag="xT")
                            if dr == "f":
                                nc.sync.dma_start(xT[:, 0:4, :], own_ap)
                                nc.sync.dma_start(xT[:, 4:8, :], peer_ap)
                            else:
                                nc.sync.dma_start(xT[:, 0:4, :], peer_ap)
                                nc.sync.dma_start(xT[:, 4:8, :], own_ap)

                        # ---- input projection for 8 steps ---------------
                        wx = io.tile([128, 4, 512], fp32, tag="wx")
                        for n in range(4):
                            pp = ps2.tile([128, 512], fp32, tag="pp")
                            for k in range(kc):
                                nc.tensor.matmul(
                                    pp[:], xT[:, k], wih_t[:, k, ts(n, 512)],
                                    start=(k == 0), stop=(k == kc - 1))
                            nc.vector.scalar_tensor_tensor(
                                wx[:, n], pp[:], 1.0, bias_t[:, ts(n, 512)],
                                op0=OP.mult, op1=OP.add)

                        # h1T accumulation tile for history / feats
                        hh = io.tile([128, 4, SPB * BL], fp32, tag="hh")
                        if lay == 0:
                            hhr = io.tile([128, 4, SPB * BL], fp32, tag="hhr")

                        # ---- 8 recurrent steps --------------------------
                        for s in range(SPB):
                            hin = hA if s % 2 == 0 else hB
                            hout = hB if s % 2 == 0 else hA
                            # move this step's Wx rows to partitions 0..BL
                            wxs = io.tile([BL, 4, 512], fp32, tag="wxs", bufs=3)
                            nc.sync.dma_start(
                                wxs[:], wx[BL * s:BL * (s + 1)])
                            g_ps = ps.tile([BL, 4, 512], fp32, tag="gates")
                            for n in range(4):
                                for k in range(4):
                                    nc.tensor.matmul(
                                        g_ps[:, n], hin[:, k],
                                        whh_t[:, k, ts(n, 512)],
                                        start=(k == 0), stop=(k == 3))
                            gi = io.tile([BL, 512], fp32, tag="gi")
                            gf = io.tile([BL, 512], fp32, tag="gf")
                            gg = io.tile([BL, 512], fp32, tag="gg")
                            go = io.tile([BL, 512], fp32, tag="go")
                            for n, gt in enumerate((gi, gf, gg, go)):
                                nc.vector.scalar_tensor_tensor(
                                    gt[:], g_ps[:, n], 1.0, wxs[:, n],
                                    op0=OP.mult, op1=OP.add)
                            nc.scalar.activation(gi[:], gi[:], AF.Sigmoid)
                            nc.scalar.activation(gf[:], gf[:], AF.Sigmoid)
                            nc.scalar.activation(gg[:], gg[:], AF.Tanh)
                            nc.scalar.activation(go[:], go[:], AF.Sigmoid)
                            cflat = c_t[:].rearrange("m c p -> m (c p)")
                            nc.vector.tensor_tensor(
                                gf[:], gf[:], cflat, op=OP.mult)      # f*c
                            nc.vector.tensor_tensor(
                                gi[:], gi[:], gg[:], op=OP.mult)      # i*g
                            nc.vector.tensor_tensor(
                                cflat, gf[:], gi[:], op=OP.add)       # c new
                            th = io.tile([BL, 512], fp32, tag="th")
                            nc.scalar.activation(th[:], cflat, AF.Tanh)
                            nc.vector.tensor_tensor(
                                th[:], th[:], go[:], op=OP.mult)      # h new
                            for k in range(4):
                                tp = ps.tile([128, 4, BL], fp32, tag="htp")
                                nc.tensor.transpose(
                                    tp[:, k], th[:, ts(k, 128)],
                                    ident[:BL, :BL])
                                nc.vector.tensor_copy(hout[:, k], tp[:, k])
                            nc.vector.tensor_copy(
                                hh[:, :, BL * s:BL * (s + 1)], hout[:])
                            if lay == 0:
                                sr = SPB - 1 - s
                                nc.vector.tensor_copy(
                                    hhr[:, :, BL * sr:BL * (sr + 1)], hout[:])

                        # ---- block epilogue -----------------------------
                        if lay == 0:
                            nc.sync.dma_start(
                                hist_d[dr].ap()[ds(blk, 1)].rearrange(
                                    "x k p f -> p (x k) f"), hh[:])
                            nc.sync.dma_start(
                                histr_d[dr].ap()[ds(blk, 1)].rearrange(
                                    "x k p f -> p (x k) f"), hhr[:])
                        elif 'nofeats' in dbg:
                            nc.sync.dma_start(
                                hist_d[dr].ap()[ds(blk, 1)].rearrange(
                                    "x k p f -> p (x k) f"), hh[:])
                        else:
                            f_ps = ps.tile([C, SPB * BL], fp32, tag="fps")
                            koff = 0 if dr == "f" else 4
                            for k in range(4):
                                nc.tensor.matmul(
                                    f_ps[:], linw_sb[:, koff + k, :], hh[:, k],
                                    start=(k == 0), stop=(k == 3))
                            f_sb = io.tile([C, SPB * BL], fp32, tag="fsb")
                            nc.vector.tensor_copy(f_sb[:], f_ps[:])
                            nc.sync.dma_start(
                                fo_d[dr].ap()[ds(blk, 1)].rearrange(
                                    "x c f -> c (x f)"), f_sb[:])

            for _ph in phases:
                lstm_phase(*_ph)

    nc.compile()
    return nc


def _get_program(T=T, V=V, NCORES=NCORES, phases=((0,'f'),(0,'b'),(1,'f'),(1,'b')), dbg=()):
    key = (T, V, NCORES, phases, dbg)
    if key not in _BUILD_CACHE:
        _BUILD_CACHE[key] = _build_program(T, V, NCORES, phases, dbg)
    return _BUILD_CACHE[key]


def _host_viterbi(feats, transitions):
    """Exact fp32 Viterbi (adds + maxes only -> bit-identical to jax)."""
    feats = np.asarray(feats, np.float32)
    trans = np.asarray(transitions, np.float32)
    Bn, Tn, Cn = feats.shape
    fv = np.full((Bn, Cn), NEG, np.float32)
    fv[:, START] = 0.0
    bps = np.empty((Bn, Tn, Cn), np.int8)
    for t in range(Tn):
        scores = fv[:, None, :] + trans[None, :, :]      # [B, next, prev]
        bps[:, t] = np.argmax(scores, axis=2)
        fv = np.max(scores, axis=2) + feats[:, t]
    terminal = fv + trans[STOP][None, :]
    best = np.argmax(terminal, axis=1)
    scores_out = terminal[np.arange(Bn), best].astype(np.float32)
    paths = np.empty((Bn, Tn), np.int32)
    tag = best.copy()
    for t in range(Tn - 1, -1, -1):
        paths[:, t] = tag
        tag = bps[np.arange(Bn), t, tag].astype(np.int64)
    return scores_out, paths


def kernel(sentence, embedding, w_ih_l0, w_hh_l0, b_ih_l0, b_hh_l0,
           w_ih_l1, w_hh_l1, b_ih_l1, b_hh_l1, lin_w, lin_b,
           transitions, h0, c0):
    from concourse.bass_utils import run_bass_kernel_spmd

    sentence = np.asarray(sentence)
    f32 = lambda x: np.ascontiguousarray(np.asarray(x), dtype=np.float32)
    embedding = f32(embedding)
    lin_w, lin_b = f32(lin_w), f32(lin_b)
    transitions = f32(transitions)
    h0, c0 = f32(h0), f32(c0)

    nc = _get_program()

    # common (replicated) input arrays
    common = {"emb": embedding, "linwT": f32(lin_w.T.copy())}
    for lay, (wi, wh, bi, bh) in {0: (w_ih_l0, w_hh_l0, b_ih_l0, b_hh_l0),
                                  1: (w_ih_l1, w_hh_l1, b_ih_l1, b_hh_l1)}.items():
        wi, wh, bi, bh = f32(wi), f32(wh), f32(bi), f32(bh)
        for d_i, dr in enumerate("fb"):
            common[f"wih{lay}{dr}"] = np.ascontiguousarray(wi[d_i].T)
            common[f"whh{lay}{dr}"] = np.ascontiguousarray(wh[d_i].T)
            bsum = (bi[d_i] + bh[d_i]).astype(np.float32)
            common[f"bias{lay}{dr}"] = np.ascontiguousarray(
                np.broadcast_to(bsum[None, :], (128, 4 * HD)))

    in_maps = []
    for g in range(NCORES):
        seqs = sentence[BL * g:BL * (g + 1)].astype(np.int64)   # [BL, T]
        tokf = np.ascontiguousarray(seqs.T.reshape(T * BL, 1)).astype(np.int32)
        tokb = np.ascontiguousarray(seqs[:, ::-1].T.reshape(T * BL, 1)).astype(np.int32)
        h0T = np.ascontiguousarray(h0[:, BL * g:BL * (g + 1), :].transpose(0, 2, 1))
        c0v = np.ascontiguousarray(c0[:, BL * g:BL * (g + 1), :])
        m = dict(common)
        m.update({"tokf": tokf, "tokb": tokb, "h0T": h0T, "c0v": c0v})
        in_maps.append(m)

    res = run_bass_kernel_spmd(nc, in_maps, core_ids=list(range(NCORES)))

    # assemble features: fo{f,b} [NBLK, C, 8*BL] per core, local time order
    feats = np.empty((B, T, C), np.float32)
    for g in range(NCORES):
        r = res.results[g]
        # [blk, c, s, b] -> [b, t, c]
        ff = r["fof"].reshape(NBLK, C, SPB, BL).transpose(3, 0, 2, 1).reshape(BL, T, C)
        fb = r["fob"].reshape(NBLK, C, SPB, BL).transpose(3, 0, 2, 1).reshape(BL, T, C)
        fb = fb[:, ::-1, :]                      # bwd partial is in reversed time
        feats[BL * g:BL * (g + 1)] = ff + fb
    feats += lin_b[None, None, :]
    global _last_feats
    _last_feats = feats

    scores, paths = _host_viterbi(feats, transitions)
    return scores, paths
